# revision 52
# baseline (speedup 1.0000x reference)
"""Bidirectional Mamba layer on 8 Trainium2 NeuronCores.

Sharding: data-parallel over batch (8 batches -> 8 cores). Each core runs
both directions (fwd on x, bwd via reversed-stride reads of the same x).

Per-core algorithm per direction, d-major layout [d on partitions, t free]:
  1. uzT = in_w @ x^T                   (PE; bwd reads x with stride -1)
  2. causal depthwise conv via PE diag(conv_w[:,k]) matmuls, SiLU on ACT
  3. dblT = xp_w @ uc^T                 (PE)  -> dt / B / C rows
  4. per chunk c (lazy): delta = ln(1+exp(dt_w@dtT + dt_b)) (2 ACT ops,
     shares the natural_log_exp table with the scan's exp), w = delta*uc
  5. B/C rows broadcast to 128 partitions via PE one-hot matmuls, copied
     to wide per-state-pair SBUF tiles (ACT/DVE split)
  6. per (c, s-pair): a = exp(A[:,s]*delta) (ACT); b2 = w*Bbc (Pool TT);
     h = tensor_tensor_scan(a, b) (DVE); hc2 = h*Cbc (DVE/Pool alternating);
     psum_y += I @ hc2 (PE, fp32 accumulate; initialized with diag(D)@uc)
  7. yg = psum_y * silu(z)   (DVE, PSUM operand)
  8. out = yg^T @ out_w^T    (PE)

The two directions are emitted interleaved (engines execute their queues
in program order): dir-b's weight loads + GEMM1/conv/z chunks are spread
into dir-f's scan phase at rotation-safe points (tile pools with shared
tags double-buffer across directions), G4-f fills the transition window,
and packed single-DMA weight layouts keep the SP queue short at startup.
Host combines: out = out_f + reverse_time(out_b).
"""

import sys

sys.path.insert(0, "/opt/trn_rl_repo")

import numpy as np
import ml_dtypes

import concourse.bass as bass
import concourse.mybir as mybir
import bass_rust
from concourse import tile
from concourse.bass_utils import run_bass_kernel_spmd

BF16 = mybir.dt.bfloat16
F32 = mybir.dt.float32
AF = mybir.ActivationFunctionType
OP = mybir.AluOpType

D_MODEL = 512
D_INNER = 1024
D_STATE = 16
D_CONV = 4
DT_RANK = 32
BATCH = 8
SEQ = 1024

P = 128
NC_D = D_INNER // P   # 8 d-chunks
NC_T = SEQ // P       # 8 t-chunks
NN = SEQ // 512       # 2 psum-free blocks
NSP = D_STATE // 2    # 8 state pairs

# Engine split for the per-(c,sp) muls, tuned against the tile cost model:
# b2-mul always on Pool; hc2-mul on Pool every POOL_HC_MOD-th pair.
POOL_HC_MOD = 2


def _dir_params(nc, d):
    return {
        # packed layouts (see _prep_dir): single-DMA loads
        "inwT": nc.declare_dram_parameter(f"inwT_{d}", [P, 4 * 2 * D_INNER], BF16, isOutput=False),
        "dtwT": nc.declare_dram_parameter(f"dtwT_{d}", [DT_RANK, D_INNER], BF16, isOutput=False),
        "outwT": nc.declare_dram_parameter(f"outwT_{d}", [P, NC_D * D_MODEL], BF16, isOutput=False),
        "smallbf": nc.declare_dram_parameter(f"smallbf_{d}", [P, NC_D * 64], BF16, isOutput=False),
        "smallf32": nc.declare_dram_parameter(f"smallf32_{d}", [P, NC_D * (D_STATE + 2)], F32, isOutput=False),
        "convdiag": nc.declare_dram_parameter(f"convdiag_{d}", [P, NC_D * D_CONV * P], BF16, isOutput=False),
        "ddiag": nc.declare_dram_parameter(f"ddiag_{d}", [P, NC_D * P], BF16, isOutput=False),
        "out": nc.declare_dram_parameter(f"out_{d}", [SEQ, D_MODEL], F32, isOutput=True),
    }


class Shared:
    pass


def _build_shared(nc, tc, pools):
    """Pools + direction-independent tiles (identity, oht, x)."""
    sh = Shared()
    sh.xT_d = nc.declare_dram_parameter("xT", [D_MODEL, SEQ], BF16, isOutput=False)
    sh.oht_d = nc.declare_dram_parameter("oht", [2 * D_STATE, 2 * D_STATE * P], BF16, isOutput=False)
    sh.ident_d = nc.declare_dram_parameter("ident", [P, P], BF16, isOutput=False)

    cst = pools["cst"]
    sh.xT = [cst.tile([P, SEQ], BF16, tag=f"xT{k}", name=f"xT{k}") for k in range(4)]
    for k in range(4):
        nc.sync.dma_start(sh.xT[k][:], sh.xT_d[k * P:(k + 1) * P, :])
    sh.ident = cst.tile([P, P], BF16, tag="ident", name="ident")
    nc.sync.dma_start(sh.ident[:], sh.ident_d[:])
    return sh


class Dir:
    """Emission helpers for one direction; methods are called in a global
    interleaved order by build_bass (engines execute in program order)."""

    def __init__(self, nc, pools, sh, p, rev, hc_pool=(0, 2, 4, 6), gate_act=False):
        self.nc, self.pools, self.sh, self.p, self.rev = nc, pools, sh, p, rev
        self.hc_pool = hc_pool
        self.gate_act = gate_act
        self.uc = [None] * NC_D
        self.sz = [None] * NC_D
        self.yg = [None] * NC_D

    def xs(self, n):
        sh, rev = self.sh, self.rev
        if not rev:
            return [sh.xT[k][:, n * 512:(n + 1) * 512] for k in range(4)]
        start = SEQ - 1 - n * 512
        stop = SEQ - 1 - (n + 1) * 512
        if stop < 0:
            return [sh.xT[k][:, start::-1] for k in range(4)]
        return [sh.xT[k][:, start:stop:-1] for k in range(4)]

    def loads(self):
        nc, p, wpool = self.nc, self.p, self.pools["w"]
        inwT_t = wpool.tile([P, 4 * 2 * D_INNER], BF16, tag="inwT", name="inwT")
        for k in range(4):
            nc.sync.dma_start(inwT_t[:, k * 2 * D_INNER:(k + 1) * 2 * D_INNER],
                              p["inwT"][:, k * 2 * D_INNER:(k + 1) * 2 * D_INNER])
        self.inwT = [inwT_t[:, k * 2 * D_INNER:(k + 1) * 2 * D_INNER] for k in range(4)]
        smallbf = wpool.tile([P, NC_D * 64], BF16, tag="smallbf", name="smallbf")
        nc.sync.dma_start(smallbf[:], p["smallbf"][:])
        smallf32 = self.pools["wsmall"].tile([P, NC_D * (D_STATE + 2)], F32, tag="smallf32", name="smallf32")
        nc.sync.dma_start(smallf32[:], p["smallf32"][:])
        W = D_STATE + 2
        self.xpwT = [smallbf[:, c * 64:(c + 1) * 64] for c in range(NC_D)]
        self.A_sb = [smallf32[:, c * W:c * W + D_STATE] for c in range(NC_D)]
        self.convb = [smallf32[:, c * W + D_STATE:c * W + D_STATE + 1] for c in range(NC_D)]
        self.dtb = [smallf32[:, c * W + D_STATE + 1:c * W + D_STATE + 2] for c in range(NC_D)]
        self.dtwT = self.pools["wsmall"].tile([DT_RANK, D_INNER], BF16, tag="dtwT", name="dtwT")
        nc.sync.dma_start(self.dtwT[:], p["dtwT"][:])

    def uconv_chunk(self, c):
        nc, p, pools = self.nc, self.p, self.pools
        ps_small = pools["ps_small"]
        uT = pools["uT"].tile([P, SEQ + D_CONV - 1], BF16, tag="uT", name=f"uT{c}")
        nc.vector.memset(uT[:, 0:D_CONV - 1], 0.0)
        for n in range(NN):
            pt = ps_small.tile([P, 512], F32, tag="g1", name="g1u")
            for k in range(4):
                nc.tensor.matmul(
                    pt[:], self.inwT[k][:, c * P:(c + 1) * P], self.xs(n)[k],
                    start=(k == 0), stop=(k == 3),
                )
            if self.rev:
                nc.vector.tensor_copy(uT[:, D_CONV - 1 + n * 512:D_CONV - 1 + (n + 1) * 512], pt[:])
            else:
                nc.scalar.copy(uT[:, D_CONV - 1 + n * 512:D_CONV - 1 + (n + 1) * 512], pt[:])
        cdt = pools["cdiag"].tile([P, D_CONV * P], BF16, tag="cdiag", name=f"cd{c}")
        nc.sync.dma_start(cdt[:], p["convdiag"][:, c * D_CONV * P:(c + 1) * D_CONV * P])
        cd = [cdt[:, k * P:(k + 1) * P] for k in range(D_CONV)]
        self.uc[c] = pools["uc"].tile([P, SEQ], BF16, tag="uc", name=f"uc{c}")
        for n in range(NN):
            pt = ps_small.tile([P, 512], F32, tag="g1", name="g1c")
            for k in range(D_CONV):
                nc.tensor.matmul(
                    pt[:], cd[k], uT[:, k + n * 512:k + n * 512 + 512],
                    start=(k == 0), stop=(k == D_CONV - 1),
                )
            nc.scalar.activation(
                self.uc[c][:, n * 512:(n + 1) * 512], pt[:], AF.Silu, bias=self.convb[c]
            )

    def z_chunk(self, c):
        nc, pools = self.nc, self.pools
        ps_small = pools["ps_small"]
        self.sz[c] = pools["sz"].tile([P, SEQ], BF16, tag="sz", name=f"sz{c}")
        for n in range(NN):
            pt = ps_small.tile([P, 512], F32, tag="g1", name="g1z")
            for k in range(4):
                nc.tensor.matmul(
                    pt[:], self.inwT[k][:, D_INNER + c * P:D_INNER + (c + 1) * P],
                    self.xs(n)[k], start=(k == 0), stop=(k == 3),
                )
            nc.scalar.activation(self.sz[c][:, n * 512:(n + 1) * 512], pt[:], AF.Silu)

    def g2(self):
        nc, pools = self.nc, self.pools
        ps_small = pools["ps_small"]
        self.dt_bf = pools["dtbf"].tile([DT_RANK, SEQ], BF16, tag="dt_bf", name="dt_bf")
        self.bc_bf = pools["bcbf"].tile([2 * D_STATE, SEQ], BF16, tag="bc_bf", name="bc_bf")
        for n in range(NN):
            pt = ps_small.tile([64, 512], F32, tag="g1", name="g2")
            for c in range(NC_D):
                nc.tensor.matmul(
                    pt[:], self.xpwT[c], self.uc[c][:, n * 512:(n + 1) * 512],
                    start=(c == 0), stop=(c == NC_D - 1),
                )
            nc.vector.tensor_copy(self.dt_bf[:, n * 512:(n + 1) * 512], pt[0:DT_RANK, :])
            nc.vector.tensor_copy(self.bc_bf[:, n * 512:(n + 1) * 512], pt[DT_RANK:64, :])

    def bcast(self, desc=False, dve_mod=2):
        nc, pools, sh = self.nc, self.pools, self.sh
        ps_small = pools["ps_small"]
        self.Bbc = [pools["bc"].tile([P, 2048], BF16, tag=f"Bbc{sp}", name=f"Bbc{sp}") for sp in range(NSP)]
        self.Cbc = [pools["bc"].tile([P, 2048], BF16, tag=f"Cbc{sp}", name=f"Cbc{sp}") for sp in range(NSP)]
        for sp in (range(NSP - 1, -1, -1) if desc else range(NSP)):
            ohs = pools["oht"].tile([2 * D_STATE, 4 * P], BF16, tag="oht", name=f"oh{sp}")
            nc.sync.dma_start(
                ohs[:].rearrange("r (h q) -> r h q", h=2),
                sh.oht_d[:].rearrange("r (h q) -> r h q", h=2)[:, :, sp * 2 * P:(sp * 2 + 2) * P],
            )
            cpy = 0
            for half, dst in ((0, self.Bbc[sp]), (1, self.Cbc[sp])):
                for j in range(2):
                    for n in range(NN):
                        pt = ps_small.tile([P, 512], F32, tag="g1", name="bcb")
                        nc.tensor.matmul(
                            pt[:], ohs[:, (half * 2 + j) * P:(half * 2 + j + 1) * P],
                            self.bc_bf[:, n * 512:(n + 1) * 512],
                            start=True, stop=True,
                        )
                        if cpy % dve_mod == 0:
                            nc.scalar.copy(
                                dst[:, j * 1024 + n * 512:j * 1024 + (n + 1) * 512], pt[:]
                            )
                        else:
                            nc.vector.tensor_copy(
                                dst[:, j * 1024 + n * 512:j * 1024 + (n + 1) * 512], pt[:]
                            )
                        cpy += 1

    def scan_prep(self):
        nc, pools = self.nc, self.pools
        self.ddt = pools["ddiag"].tile([P, NC_D * P], BF16, tag="ddiag", name="ddiag")
        nc.sync.dma_start(self.ddt[:], self.p["ddiag"][:])

    def scan_chunk_prep(self, c):
        nc, pools = self.nc, self.pools
        ps_small = pools["ps_small"]
        pt = ps_small.tile([P, 512], F32, tag="g1", name="g3a")
        pt2 = ps_small.tile([P, 512], F32, tag="g1", name="g3b")
        nc.tensor.matmul(pt[:], self.dtwT[:, c * P:(c + 1) * P], self.dt_bf[:, 0:512],
                         start=True, stop=True)
        nc.tensor.matmul(pt2[:], self.dtwT[:, c * P:(c + 1) * P], self.dt_bf[:, 512:1024],
                         start=True, stop=True)
        delta = pools["delta"].tile([P, SEQ], BF16, tag="delta", name=f"delta{c}")
        nc.scalar.activation(delta[:, 0:512], pt[:], AF.Exp, bias=self.dtb[c])
        nc.scalar.activation(delta[:, 512:1024], pt2[:], AF.Exp, bias=self.dtb[c])
        nc.scalar.activation(delta[:], delta[:], AF.Ln, bias=1.0)
        w_t = pools["wt"].tile([P, SEQ], BF16, tag="wt", name=f"w{c}")
        nc.vector.tensor_mul(w_t[:], delta[:], self.uc[c][:])
        self._prep = (delta, w_t)

    def scan_chunk_body(self, c, desc=False):
        nc, pools, sh = self.nc, self.pools, self.sh
        ps_y = pools["ps_y"]
        delta, w_t = self._chunk_prep[c]
        w_b = w_t[:].unsqueeze(1).broadcast_to((P, 2, 1024))

        py = ps_y.tile([P, SEQ], F32, tag="py", name=f"py{c}")
        for n in range(NN):
            nc.tensor.matmul(py[:, n * 512:(n + 1) * 512], self.ddt[:, c * P:(c + 1) * P],
                             self.uc[c][:, n * 512:(n + 1) * 512], start=True, stop=False)

        sps = range(NSP - 1, -1, -1) if desc else range(NSP)
        for i, sp in enumerate(sps):
            a2 = pools["a2"].tile([P, 2048], BF16, tag="a2", name="a2")
            for j in range(2):
                nc.scalar.activation(
                    a2[:, j * 1024:(j + 1) * 1024], delta[:], AF.Exp,
                    scale=self.A_sb[c][:, sp * 2 + j:sp * 2 + j + 1],
                )
            b2 = pools["b2"].tile([P, 2048], BF16, tag="b2", name="b2")
            nc.gpsimd.tensor_tensor(
                b2[:].rearrange("p (s n) -> p s n", s=2), w_b,
                self.Bbc[sp][:].rearrange("p (s n) -> p s n", s=2), OP.mult,
            )
            h2 = pools["h2"].tile([P, 2048], BF16, tag="h2", name="h2")
            for j in range(2):
                nc.vector.tensor_tensor_scan(
                    h2[:, j * 1024:(j + 1) * 1024],
                    a2[:, j * 1024:(j + 1) * 1024],
                    b2[:, j * 1024:(j + 1) * 1024],
                    0.0, op0=OP.mult, op1=OP.add,
                )
            hc2 = pools["hc2"].tile([P, 2048], BF16, tag="hc2", name="hc2")
            hmul = nc.gpsimd if i % NSP in self.hc_pool else nc.vector
            hmul.tensor_mul(hc2[:], h2[:], self.Cbc[sp][:])
            last = i == NSP - 1
            for j in range(2):
                for n in range(NN):
                    nc.tensor.matmul(
                        py[:, n * 512:(n + 1) * 512], sh.ident[:],
                        hc2[:, j * 1024 + n * 512:j * 1024 + (n + 1) * 512],
                        start=False, stop=(last and j == 1),
                    )
        self.yg[c] = pools["yg"].tile([P, SEQ], BF16, tag="yg", name=f"yg{c}")
        if self.gate_act:
            tmp = pools["wt"].tile([P, SEQ], BF16, tag="wt", name=f"gt{c}")
            nc.scalar.copy(tmp[:], py[:])
            nc.vector.tensor_mul(self.yg[c][:], tmp[:], self.sz[c][:])
        else:
            nc.vector.tensor_mul(self.yg[c][:], py[:], self.sz[c][:])

    def prep(self, c):
        self.scan_chunk_prep(c)
        if not hasattr(self, "_chunk_prep"):
            self._chunk_prep = {}
        self._chunk_prep[c] = self._prep

    def g4_prep(self):
        nc, wpool = self.nc, self.pools["w"]
        outwT_t = wpool.tile([P, NC_D * D_MODEL], BF16, tag="outwT", name="outwT")
        nc.sync.dma_start(outwT_t[:], self.p["outwT"][:])
        self.outwT = [outwT_t[:, c * D_MODEL:(c + 1) * D_MODEL] for c in range(NC_D)]

    def g4_block(self, m):
        nc, pools = self.nc, self.pools
        pt = pools["ps_small"].tile([P, D_MODEL], F32, tag="g1", name="g4")
        for c in range(NC_D):
            nc.tensor.matmul(
                pt[:], self.yg[c][:, m * P:(m + 1) * P], self.outwT[c],
                start=(c == 0), stop=(c == NC_D - 1),
            )
        ot = pools["g4o"].tile([P, D_MODEL], BF16, tag="g4o", name="ot")
        nc.scalar.copy(ot[:], pt[:])
        nc.gpsimd.dma_start(self.p["out"][m * P:(m + 1) * P, :], ot[:])


def _split_excess_waits(nc):
    """walrus accepts at most one sync-wait per instruction (two for
    EventSemaphore); hoist the excess onto injected same-engine NoOps."""
    for f in nc.m.functions:
        for bb in f.blocks:
            new_insts = []
            for inst in bb.instructions:
                si = inst.sync_info
                cap = 2 if isinstance(inst, mybir.InstEventSemaphore) else 1
                if si is not None and len(si.on_wait) > cap:
                    waits = list(si.on_wait)
                    for i, w in enumerate(waits[:-cap]):
                        nop = mybir.InstNoOp(
                            name=f"{inst.name}-wsplit{i}", ins=[], outs=[]
                        )
                        nop.engine = inst.engine
                        nop.sync_info = bass_rust.SyncInfo(on_wait=[w], on_update=[])
                        new_insts.append(nop)
                    inst.sync_info = bass_rust.SyncInfo(
                        on_wait=waits[-cap:], on_update=list(si.on_update)
                    )
                new_insts.append(inst)
            try:
                bb.instructions = new_insts
            except Exception:
                bb.instructions.clear()
                bb.instructions.extend(new_insts)


def build_bass():
    nc = bass.Bass()
    params = {d: _dir_params(nc, d) for d in ("f", "b")}
    with tile.TileContext(nc) as tc:
        import contextlib
        with contextlib.ExitStack() as st:
            pools = {
                "cst": st.enter_context(tc.tile_pool(name="cst", bufs=1)),
                "w": st.enter_context(tc.tile_pool(name="w", bufs=1)),
                "wsmall": st.enter_context(tc.tile_pool(name="wsmall", bufs=2)),
                "uT": st.enter_context(tc.tile_pool(name="uT", bufs=2)),
                "uc": st.enter_context(tc.tile_pool(name="uc", bufs=8)),
                "sz": st.enter_context(tc.tile_pool(name="sz", bufs=8)),
                "yg": st.enter_context(tc.tile_pool(name="yg", bufs=8)),
                "delta": st.enter_context(tc.tile_pool(name="delta", bufs=2)),
                "wt": st.enter_context(tc.tile_pool(name="wt", bufs=2)),
                "dtbf": st.enter_context(tc.tile_pool(name="dtbf", bufs=1)),
                "bcbf": st.enter_context(tc.tile_pool(name="bcbf", bufs=1)),
                "bc": st.enter_context(tc.tile_pool(name="bc", bufs=1)),
                "cdiag": st.enter_context(tc.tile_pool(name="cdiag", bufs=2)),
                "ddiag": st.enter_context(tc.tile_pool(name="ddiag", bufs=1)),
                "oht": st.enter_context(tc.tile_pool(name="oht", bufs=1)),
                "a2": st.enter_context(tc.tile_pool(name="a2", bufs=2)),
                "b2": st.enter_context(tc.tile_pool(name="b2", bufs=2)),
                "h2": st.enter_context(tc.tile_pool(name="h2", bufs=2)),
                "hc2": st.enter_context(tc.tile_pool(name="hc2", bufs=2)),
                "g4o": st.enter_context(tc.tile_pool(name="g4o", bufs=2)),
                "ps_small": st.enter_context(tc.tile_pool(name="ps_small", bufs=4, space="PSUM")),
                "ps_y": st.enter_context(tc.tile_pool(name="ps_y", bufs=2, space="PSUM")),
            }
            sh = _build_shared(nc, tc, pools)
            f = Dir(nc, pools, sh, params["f"], rev=False, hc_pool=(0, 2, 4, 6))
            b = Dir(nc, pools, sh, params["b"], rev=True, hc_pool=(0, 2, 4, 6))
            f.loads()
            for c in range(NC_D):
                f.uconv_chunk(c)
            f.g2()
            f.scan_prep()
            f.prep(0)
            f.prep(1)
            f.bcast()
            for c in range(NC_D):
                f.z_chunk(c)
            b.loads()
            f.scan_chunk_body(0)
            f.prep(2)
            f.scan_chunk_body(1)
            f.prep(3)
            f.scan_chunk_body(2)
            f.prep(4)
            f.scan_chunk_body(3)
            for c in range(0, 2):
                b.uconv_chunk(c)
            for c in range(0, 2):
                b.z_chunk(c)
            f.prep(5)
            f.scan_chunk_body(4)
            for c in range(2, 4):
                b.uconv_chunk(c)
            for c in range(2, 4):
                b.z_chunk(c)
            b.uconv_chunk(4)
            b.z_chunk(4)
            f.prep(6)
            f.scan_chunk_body(5)
            b.uconv_chunk(5)
            b.z_chunk(5)
            f.prep(7)
            f.scan_chunk_body(6)
            b.uconv_chunk(6)
            b.z_chunk(6)
            f.scan_chunk_body(7)
            b.uconv_chunk(7)
            b.z_chunk(7)
            b.g2()
            b.scan_prep()
            b.prep(0)
            b.prep(1)
            b.bcast(dve_mod=2)
            f.g4_prep()
            for m in range(NC_T):
                f.g4_block(m)
            b.g4_prep()
            b.scan_chunk_body(0)
            b.prep(2)
            b.scan_chunk_body(1)
            b.prep(3)
            b.scan_chunk_body(2)
            b.prep(4)
            b.scan_chunk_body(3)
            b.prep(5)
            b.scan_chunk_body(4)
            b.prep(6)
            b.scan_chunk_body(5)
            b.prep(7)
            b.scan_chunk_body(6)
            b.scan_chunk_body(7)
            for m in range(NC_T):
                b.g4_block(m)
    _split_excess_waits(nc)
    return nc


def _prep_dir(w):
    bf = ml_dtypes.bfloat16
    in_w, conv_w, conv_b, xp_w, dt_w, dt_b, A_log, Dp, out_w = w
    in_wT = np.asarray(in_w, np.float32).T            # [512, 2048]
    out_wT = np.asarray(out_w, np.float32).T          # [1024, 512]
    xp_wT = np.asarray(xp_w, np.float32).T            # [1024, 64]
    conv_w = np.asarray(conv_w, np.float32)
    conv_b = np.asarray(conv_b, np.float32)
    dt_b = np.asarray(dt_b, np.float32)
    A = -np.exp(np.asarray(A_log, np.float64)).astype(np.float32)
    Dp = np.asarray(Dp, np.float32)

    # inwT packed [P, 4*2048]: k-block k holds in_wT rows k*128..k*128+127
    inwT = in_wT.reshape(4, P, 2 * D_INNER).transpose(1, 0, 2).reshape(P, 4 * 2 * D_INNER)
    # outwT packed [P, 8*512]: c-block holds out_wT rows c*128..
    outwT = out_wT.reshape(NC_D, P, D_MODEL).transpose(1, 0, 2).reshape(P, NC_D * D_MODEL)
    # smallbf [P, 8*64]: xp_wT rows per chunk
    smallbf = xp_wT.reshape(NC_D, P, 64).transpose(1, 0, 2).reshape(P, NC_D * 64)
    # smallf32 [P, 8*18]: per chunk [A(16) | conv_b | dt_b]
    sf = np.concatenate(
        [A.reshape(NC_D, P, D_STATE),
         conv_b.reshape(NC_D, P, 1),
         dt_b.reshape(NC_D, P, 1)], axis=2)
    smallf32 = sf.transpose(1, 0, 2).reshape(P, NC_D * (D_STATE + 2))
    # convdiag [P, 8*4*128]: block (c,k) = diag(conv_w[c*128: , k])
    convdiag = np.zeros((P, NC_D * D_CONV * P), np.float32)
    for c in range(NC_D):
        for k in range(D_CONV):
            blk = (c * D_CONV + k) * P
            convdiag[:, blk:blk + P] = np.diag(conv_w[c * P:(c + 1) * P, k])
    # ddiag [P, 8*128]
    ddiag = np.zeros((P, NC_D * P), np.float32)
    for c in range(NC_D):
        ddiag[:, c * P:(c + 1) * P] = np.diag(Dp[c * P:(c + 1) * P])
    return {
        "inwT": np.ascontiguousarray(inwT).astype(bf),
        "dtwT": np.ascontiguousarray(np.asarray(dt_w).T).astype(bf),
        "outwT": np.ascontiguousarray(outwT).astype(bf),
        "smallbf": np.ascontiguousarray(smallbf).astype(bf),
        "smallf32": np.ascontiguousarray(smallf32),
        "convdiag": np.ascontiguousarray(convdiag).astype(bf),
        "ddiag": np.ascontiguousarray(ddiag).astype(bf),
    }


_CACHED = {}


def kernel(
    x,
    in_w_f, conv_w_f, conv_b_f, xp_w_f, dt_w_f, dt_b_f, A_log_f, D_f, out_w_f,
    in_w_b, conv_w_b, conv_b_b, xp_w_b, dt_w_b, dt_b_b, A_log_b, D_b, out_w_b,
):
    bf = ml_dtypes.bfloat16
    x = np.asarray(x, dtype=np.float32)

    if "nc" not in _CACHED:
        _CACHED["nc"] = build_bass()
    nc = _CACHED["nc"]

    wf = _prep_dir((in_w_f, conv_w_f, conv_b_f, xp_w_f, dt_w_f, dt_b_f,
                    A_log_f, D_f, out_w_f))
    wb = _prep_dir((in_w_b, conv_w_b, conv_b_b, xp_w_b, dt_w_b, dt_b_b,
                    A_log_b, D_b, out_w_b))
    oht = np.kron(np.eye(2 * D_STATE, dtype=np.float32),
                  np.ones((1, P), np.float32)).astype(bf)
    ident = np.eye(P, dtype=np.float32).astype(bf)

    in_maps = []
    for b in range(BATCH):
        m = {"oht": oht, "ident": ident}
        for d, wd in (("f", wf), ("b", wb)):
            for k, v in wd.items():
                m[f"{k}_{d}"] = v
        m["xT"] = np.ascontiguousarray(x[b].T).astype(bf)
        in_maps.append(m)

    res = run_bass_kernel_spmd(nc, in_maps, core_ids=list(range(BATCH)))
    out = np.empty((BATCH, SEQ, D_MODEL), np.float32)
    for b in range(BATCH):
        rb = res.results[b]
        out[b] = rb["out_f"] + rb["out_b"][::-1]
    return out


# revision 53
# speedup vs baseline: 1.0050x; 1.0050x over previous
"""Bidirectional Mamba layer on 8 Trainium2 NeuronCores.

Sharding: data-parallel over batch (8 batches -> 8 cores). Each core runs
both directions (fwd on x, bwd via reversed-stride reads of the same x).

Per-core algorithm per direction, d-major layout [d on partitions, t free]:
  1. uzT = in_w @ x^T                   (PE; bwd reads x with stride -1)
  2. causal depthwise conv via PE diag(conv_w[:,k]) matmuls, SiLU on ACT
  3. dblT = xp_w @ uc^T                 (PE)  -> dt / B / C rows
  4. per chunk c (lazy): delta = ln(1+exp(dt_w@dtT + dt_b)) (2 ACT ops,
     shares the natural_log_exp table with the scan's exp), w = delta*uc
  5. B/C rows broadcast to 128 partitions via PE one-hot matmuls, copied
     to wide per-state-pair SBUF tiles (ACT/DVE split)
  6. per (c, s-pair): a = exp(A[:,s]*delta) (ACT); b2 = w*Bbc (Pool TT);
     h = tensor_tensor_scan(a, b) (DVE); hc2 = h*Cbc (DVE/Pool alternating);
     psum_y += I @ hc2 (PE, fp32 accumulate; initialized with diag(D)@uc)
  7. yg = psum_y * silu(z)   (DVE, PSUM operand)
  8. out = yg^T @ out_w^T    (PE)

The two directions are emitted interleaved (engines execute their queues
in program order): dir-b's weight loads + GEMM1/conv/z chunks are spread
into dir-f's scan phase at rotation-safe points (tile pools with shared
tags double-buffer across directions), G4-f fills the transition window,
and packed single-DMA weight layouts keep the SP queue short at startup.
Host combines: out = out_f + reverse_time(out_b).
"""

import sys

sys.path.insert(0, "/opt/trn_rl_repo")

import numpy as np
import ml_dtypes

import concourse.bass as bass
import concourse.mybir as mybir
import bass_rust
from concourse import tile
from concourse.bass_utils import run_bass_kernel_spmd

BF16 = mybir.dt.bfloat16
F32 = mybir.dt.float32
AF = mybir.ActivationFunctionType
OP = mybir.AluOpType

D_MODEL = 512
D_INNER = 1024
D_STATE = 16
D_CONV = 4
DT_RANK = 32
BATCH = 8
SEQ = 1024

P = 128
NC_D = D_INNER // P   # 8 d-chunks
NC_T = SEQ // P       # 8 t-chunks
NN = SEQ // 512       # 2 psum-free blocks
NSP = D_STATE // 2    # 8 state pairs

# Engine split for the per-(c,sp) muls, tuned against the tile cost model:
# b2-mul always on Pool; hc2-mul on Pool every POOL_HC_MOD-th pair.
POOL_HC_MOD = 2


def _dir_params(nc, d):
    return {
        # packed layouts (see _prep_dir): single-DMA loads
        "inwT": nc.declare_dram_parameter(f"inwT_{d}", [P, 4 * 2 * D_INNER], BF16, isOutput=False),
        "dtwT": nc.declare_dram_parameter(f"dtwT_{d}", [DT_RANK, D_INNER], BF16, isOutput=False),
        "outwT": nc.declare_dram_parameter(f"outwT_{d}", [P, NC_D * D_MODEL], BF16, isOutput=False),
        "smallbf": nc.declare_dram_parameter(f"smallbf_{d}", [P, NC_D * 64], BF16, isOutput=False),
        "smallf32": nc.declare_dram_parameter(f"smallf32_{d}", [P, NC_D * (D_STATE + 2)], F32, isOutput=False),
        "convdiag": nc.declare_dram_parameter(f"convdiag_{d}", [P, NC_D * D_CONV * P], BF16, isOutput=False),
        "ddiag": nc.declare_dram_parameter(f"ddiag_{d}", [P, NC_D * P], BF16, isOutput=False),
        "out": nc.declare_dram_parameter(f"out_{d}", [SEQ, D_MODEL], F32, isOutput=True),
    }


class Shared:
    pass


def _build_shared(nc, tc, pools):
    """Pools + direction-independent tiles (identity, oht, x)."""
    sh = Shared()
    sh.xT_d = nc.declare_dram_parameter("xT", [D_MODEL, SEQ], BF16, isOutput=False)
    sh.oht_d = nc.declare_dram_parameter("oht", [2 * D_STATE, 2 * D_STATE * P], BF16, isOutput=False)
    sh.ident_d = nc.declare_dram_parameter("ident", [P, P], BF16, isOutput=False)

    cst = pools["cst"]
    sh.xT = [cst.tile([P, SEQ], BF16, tag=f"xT{k}", name=f"xT{k}") for k in range(4)]
    for k in range(4):
        nc.sync.dma_start(sh.xT[k][:], sh.xT_d[k * P:(k + 1) * P, :])
    sh.ident = cst.tile([P, P], BF16, tag="ident", name="ident")
    nc.sync.dma_start(sh.ident[:], sh.ident_d[:])
    return sh


class Dir:
    """Emission helpers for one direction; methods are called in a global
    interleaved order by build_bass (engines execute in program order)."""

    def __init__(self, nc, pools, sh, p, rev, hc_pool=(0, 2, 4, 6), gate_act=False):
        self.nc, self.pools, self.sh, self.p, self.rev = nc, pools, sh, p, rev
        self.hc_pool = hc_pool
        self.gate_act = gate_act
        self.uc = [None] * NC_D
        self.sz = [None] * NC_D
        self.yg = [None] * NC_D

    def xs(self, n):
        sh, rev = self.sh, self.rev
        if not rev:
            return [sh.xT[k][:, n * 512:(n + 1) * 512] for k in range(4)]
        start = SEQ - 1 - n * 512
        stop = SEQ - 1 - (n + 1) * 512
        if stop < 0:
            return [sh.xT[k][:, start::-1] for k in range(4)]
        return [sh.xT[k][:, start:stop:-1] for k in range(4)]

    def loads(self):
        nc, p, wpool = self.nc, self.p, self.pools["w"]
        inwT_t = wpool.tile([P, 4 * 2 * D_INNER], BF16, tag="inwT", name="inwT")
        for k in range(4):
            nc.sync.dma_start(inwT_t[:, k * 2 * D_INNER:(k + 1) * 2 * D_INNER],
                              p["inwT"][:, k * 2 * D_INNER:(k + 1) * 2 * D_INNER])
        self.inwT = [inwT_t[:, k * 2 * D_INNER:(k + 1) * 2 * D_INNER] for k in range(4)]
        smallbf = wpool.tile([P, NC_D * 64], BF16, tag="smallbf", name="smallbf")
        nc.sync.dma_start(smallbf[:], p["smallbf"][:])
        smallf32 = self.pools["wsmall"].tile([P, NC_D * (D_STATE + 2)], F32, tag="smallf32", name="smallf32")
        nc.sync.dma_start(smallf32[:], p["smallf32"][:])
        W = D_STATE + 2
        self.xpwT = [smallbf[:, c * 64:(c + 1) * 64] for c in range(NC_D)]
        self.A_sb = [smallf32[:, c * W:c * W + D_STATE] for c in range(NC_D)]
        self.convb = [smallf32[:, c * W + D_STATE:c * W + D_STATE + 1] for c in range(NC_D)]
        self.dtb = [smallf32[:, c * W + D_STATE + 1:c * W + D_STATE + 2] for c in range(NC_D)]
        self.dtwT = self.pools["wsmall"].tile([DT_RANK, D_INNER], BF16, tag="dtwT", name="dtwT")
        nc.sync.dma_start(self.dtwT[:], p["dtwT"][:])

    def uconv_chunk(self, c):
        nc, p, pools = self.nc, self.p, self.pools
        ps_small = pools["ps_small"]
        uT = pools["uT"].tile([P, SEQ + D_CONV - 1], BF16, tag="uT", name=f"uT{c}")
        nc.vector.memset(uT[:, 0:D_CONV - 1], 0.0)
        for n in range(NN):
            pt = ps_small.tile([P, 512], F32, tag="g1", name="g1u")
            for k in range(4):
                nc.tensor.matmul(
                    pt[:], self.inwT[k][:, c * P:(c + 1) * P], self.xs(n)[k],
                    start=(k == 0), stop=(k == 3),
                )
            if self.rev:
                nc.vector.tensor_copy(uT[:, D_CONV - 1 + n * 512:D_CONV - 1 + (n + 1) * 512], pt[:])
            else:
                nc.scalar.copy(uT[:, D_CONV - 1 + n * 512:D_CONV - 1 + (n + 1) * 512], pt[:])
        cdt = pools["cdiag"].tile([P, D_CONV * P], BF16, tag="cdiag", name=f"cd{c}")
        nc.sync.dma_start(cdt[:], p["convdiag"][:, c * D_CONV * P:(c + 1) * D_CONV * P])
        cd = [cdt[:, k * P:(k + 1) * P] for k in range(D_CONV)]
        self.uc[c] = pools["uc"].tile([P, SEQ], BF16, tag="uc", name=f"uc{c}")
        for n in range(NN):
            pt = ps_small.tile([P, 512], F32, tag="g1", name="g1c")
            for k in range(D_CONV):
                nc.tensor.matmul(
                    pt[:], cd[k], uT[:, k + n * 512:k + n * 512 + 512],
                    start=(k == 0), stop=(k == D_CONV - 1),
                )
            nc.scalar.activation(
                self.uc[c][:, n * 512:(n + 1) * 512], pt[:], AF.Silu, bias=self.convb[c]
            )

    def z_chunk(self, c):
        nc, pools = self.nc, self.pools
        ps_small = pools["ps_small"]
        self.sz[c] = pools["sz"].tile([P, SEQ], BF16, tag="sz", name=f"sz{c}")
        for n in range(NN):
            pt = ps_small.tile([P, 512], F32, tag="g1", name="g1z")
            for k in range(4):
                nc.tensor.matmul(
                    pt[:], self.inwT[k][:, D_INNER + c * P:D_INNER + (c + 1) * P],
                    self.xs(n)[k], start=(k == 0), stop=(k == 3),
                )
            nc.scalar.activation(self.sz[c][:, n * 512:(n + 1) * 512], pt[:], AF.Silu)

    def g2(self):
        nc, pools = self.nc, self.pools
        ps_small = pools["ps_small"]
        self.dt_bf = pools["dtbf"].tile([DT_RANK, SEQ], BF16, tag="dt_bf", name="dt_bf")
        self.bc_bf = pools["bcbf"].tile([2 * D_STATE, SEQ], BF16, tag="bc_bf", name="bc_bf")
        for n in range(NN):
            pt = ps_small.tile([64, 512], F32, tag="g1", name="g2")
            for c in range(NC_D):
                nc.tensor.matmul(
                    pt[:], self.xpwT[c], self.uc[c][:, n * 512:(n + 1) * 512],
                    start=(c == 0), stop=(c == NC_D - 1),
                )
            nc.vector.tensor_copy(self.dt_bf[:, n * 512:(n + 1) * 512], pt[0:DT_RANK, :])
            nc.vector.tensor_copy(self.bc_bf[:, n * 512:(n + 1) * 512], pt[DT_RANK:64, :])

    def bcast(self, desc=False, dve_mod=2):
        nc, pools, sh = self.nc, self.pools, self.sh
        ps_small = pools["ps_small"]
        self.Bbc = [pools["bc"].tile([P, 2048], BF16, tag=f"Bbc{sp}", name=f"Bbc{sp}") for sp in range(NSP)]
        self.Cbc = [pools["bc"].tile([P, 2048], BF16, tag=f"Cbc{sp}", name=f"Cbc{sp}") for sp in range(NSP)]
        for sp in (range(NSP - 1, -1, -1) if desc else range(NSP)):
            ohs = pools["oht"].tile([2 * D_STATE, 4 * P], BF16, tag="oht", name=f"oh{sp}")
            nc.sync.dma_start(
                ohs[:].rearrange("r (h q) -> r h q", h=2),
                sh.oht_d[:].rearrange("r (h q) -> r h q", h=2)[:, :, sp * 2 * P:(sp * 2 + 2) * P],
            )
            cpy = 0
            for half, dst in ((0, self.Bbc[sp]), (1, self.Cbc[sp])):
                for j in range(2):
                    for n in range(NN):
                        pt = ps_small.tile([P, 512], F32, tag="g1", name="bcb")
                        nc.tensor.matmul(
                            pt[:], ohs[:, (half * 2 + j) * P:(half * 2 + j + 1) * P],
                            self.bc_bf[:, n * 512:(n + 1) * 512],
                            start=True, stop=True,
                        )
                        if cpy % dve_mod == 0:
                            nc.scalar.copy(
                                dst[:, j * 1024 + n * 512:j * 1024 + (n + 1) * 512], pt[:]
                            )
                        else:
                            nc.vector.tensor_copy(
                                dst[:, j * 1024 + n * 512:j * 1024 + (n + 1) * 512], pt[:]
                            )
                        cpy += 1

    def scan_prep(self):
        nc, pools = self.nc, self.pools
        self.ddt = pools["ddiag"].tile([P, NC_D * P], BF16, tag="ddiag", name="ddiag")
        nc.sync.dma_start(self.ddt[:], self.p["ddiag"][:])

    def scan_chunk_prep(self, c):
        nc, pools = self.nc, self.pools
        ps_small = pools["ps_small"]
        pt = ps_small.tile([P, 512], F32, tag="g1", name="g3a")
        pt2 = ps_small.tile([P, 512], F32, tag="g1", name="g3b")
        nc.tensor.matmul(pt[:], self.dtwT[:, c * P:(c + 1) * P], self.dt_bf[:, 0:512],
                         start=True, stop=True)
        nc.tensor.matmul(pt2[:], self.dtwT[:, c * P:(c + 1) * P], self.dt_bf[:, 512:1024],
                         start=True, stop=True)
        delta = pools["delta"].tile([P, SEQ], BF16, tag="delta", name=f"delta{c}")
        nc.scalar.activation(delta[:, 0:512], pt[:], AF.Exp, bias=self.dtb[c])
        nc.scalar.activation(delta[:, 512:1024], pt2[:], AF.Exp, bias=self.dtb[c])
        nc.scalar.activation(delta[:], delta[:], AF.Ln, bias=1.0)
        w_t = pools["wt"].tile([P, SEQ], BF16, tag="wt", name=f"w{c}")
        nc.vector.tensor_mul(w_t[:], delta[:], self.uc[c][:])
        self._prep = (delta, w_t)

    def scan_chunk_body(self, c, desc=False):
        nc, pools, sh = self.nc, self.pools, self.sh
        ps_y = pools["ps_y"]
        delta, w_t = self._chunk_prep[c]
        w_b = w_t[:].unsqueeze(1).broadcast_to((P, 2, 1024))

        py = ps_y.tile([P, SEQ], F32, tag="py", name=f"py{c}")
        for n in range(NN):
            nc.tensor.matmul(py[:, n * 512:(n + 1) * 512], self.ddt[:, c * P:(c + 1) * P],
                             self.uc[c][:, n * 512:(n + 1) * 512], start=True, stop=False)

        sps = range(NSP - 1, -1, -1) if desc else range(NSP)
        for i, sp in enumerate(sps):
            a2 = pools["a2"].tile([P, 2048], BF16, tag="a2", name="a2")
            for j in range(2):
                nc.scalar.activation(
                    a2[:, j * 1024:(j + 1) * 1024], delta[:], AF.Exp,
                    scale=self.A_sb[c][:, sp * 2 + j:sp * 2 + j + 1],
                )
            b2 = pools["b2"].tile([P, 2048], BF16, tag="b2", name="b2")
            nc.gpsimd.tensor_tensor(
                b2[:].rearrange("p (s n) -> p s n", s=2), w_b,
                self.Bbc[sp][:].rearrange("p (s n) -> p s n", s=2), OP.mult,
            )
            h2 = pools["h2"].tile([P, 2048], BF16, tag="h2", name="h2")
            for j in range(2):
                nc.vector.tensor_tensor_scan(
                    h2[:, j * 1024:(j + 1) * 1024],
                    a2[:, j * 1024:(j + 1) * 1024],
                    b2[:, j * 1024:(j + 1) * 1024],
                    0.0, op0=OP.mult, op1=OP.add,
                )
            hc2 = pools["hc2"].tile([P, 2048], BF16, tag="hc2", name="hc2")
            hmul = nc.gpsimd if i % NSP in self.hc_pool else nc.vector
            hmul.tensor_mul(hc2[:], h2[:], self.Cbc[sp][:])
            last = i == NSP - 1
            for j in range(2):
                for n in range(NN):
                    nc.tensor.matmul(
                        py[:, n * 512:(n + 1) * 512], sh.ident[:],
                        hc2[:, j * 1024 + n * 512:j * 1024 + (n + 1) * 512],
                        start=False, stop=(last and j == 1),
                    )
        self.yg[c] = pools["yg"].tile([P, SEQ], BF16, tag="yg", name=f"yg{c}")
        if self.gate_act:
            tmp = pools["wt"].tile([P, SEQ], BF16, tag="wt", name=f"gt{c}")
            nc.scalar.copy(tmp[:], py[:])
            nc.vector.tensor_mul(self.yg[c][:], tmp[:], self.sz[c][:])
        else:
            nc.vector.tensor_mul(self.yg[c][:], py[:], self.sz[c][:])

    def prep(self, c):
        self.scan_chunk_prep(c)
        if not hasattr(self, "_chunk_prep"):
            self._chunk_prep = {}
        self._chunk_prep[c] = self._prep

    def g4_prep(self):
        nc, wpool = self.nc, self.pools["w"]
        outwT_t = wpool.tile([P, NC_D * D_MODEL], BF16, tag="outwT", name="outwT")
        nc.sync.dma_start(outwT_t[:], self.p["outwT"][:])
        self.outwT = [outwT_t[:, c * D_MODEL:(c + 1) * D_MODEL] for c in range(NC_D)]

    def g4_block(self, m):
        nc, pools = self.nc, self.pools
        pt = pools["ps_small"].tile([P, D_MODEL], F32, tag="g1", name="g4")
        for c in range(NC_D):
            nc.tensor.matmul(
                pt[:], self.yg[c][:, m * P:(m + 1) * P], self.outwT[c],
                start=(c == 0), stop=(c == NC_D - 1),
            )
        ot = pools["g4o"].tile([P, D_MODEL], BF16, tag="g4o", name="ot")
        nc.scalar.copy(ot[:], pt[:])
        nc.gpsimd.dma_start(self.p["out"][m * P:(m + 1) * P, :], ot[:])


def _split_excess_waits(nc):
    """walrus accepts at most one sync-wait per instruction (two for
    EventSemaphore); hoist the excess onto injected same-engine NoOps."""
    for f in nc.m.functions:
        for bb in f.blocks:
            new_insts = []
            for inst in bb.instructions:
                si = inst.sync_info
                cap = 2 if isinstance(inst, mybir.InstEventSemaphore) else 1
                if si is not None and len(si.on_wait) > cap:
                    waits = list(si.on_wait)
                    for i, w in enumerate(waits[:-cap]):
                        nop = mybir.InstNoOp(
                            name=f"{inst.name}-wsplit{i}", ins=[], outs=[]
                        )
                        nop.engine = inst.engine
                        nop.sync_info = bass_rust.SyncInfo(on_wait=[w], on_update=[])
                        new_insts.append(nop)
                    inst.sync_info = bass_rust.SyncInfo(
                        on_wait=waits[-cap:], on_update=list(si.on_update)
                    )
                new_insts.append(inst)
            try:
                bb.instructions = new_insts
            except Exception:
                bb.instructions.clear()
                bb.instructions.extend(new_insts)


def build_bass():
    nc = bass.Bass()
    params = {d: _dir_params(nc, d) for d in ("f", "b")}
    with tile.TileContext(nc) as tc:
        import contextlib
        with contextlib.ExitStack() as st:
            pools = {
                "cst": st.enter_context(tc.tile_pool(name="cst", bufs=1)),
                "w": st.enter_context(tc.tile_pool(name="w", bufs=1)),
                "wsmall": st.enter_context(tc.tile_pool(name="wsmall", bufs=2)),
                "uT": st.enter_context(tc.tile_pool(name="uT", bufs=2)),
                "uc": st.enter_context(tc.tile_pool(name="uc", bufs=8)),
                "sz": st.enter_context(tc.tile_pool(name="sz", bufs=8)),
                "yg": st.enter_context(tc.tile_pool(name="yg", bufs=8)),
                "delta": st.enter_context(tc.tile_pool(name="delta", bufs=2)),
                "wt": st.enter_context(tc.tile_pool(name="wt", bufs=2)),
                "dtbf": st.enter_context(tc.tile_pool(name="dtbf", bufs=1)),
                "bcbf": st.enter_context(tc.tile_pool(name="bcbf", bufs=1)),
                "bc": st.enter_context(tc.tile_pool(name="bc", bufs=1)),
                "cdiag": st.enter_context(tc.tile_pool(name="cdiag", bufs=2)),
                "ddiag": st.enter_context(tc.tile_pool(name="ddiag", bufs=1)),
                "oht": st.enter_context(tc.tile_pool(name="oht", bufs=1)),
                "a2": st.enter_context(tc.tile_pool(name="a2", bufs=2)),
                "b2": st.enter_context(tc.tile_pool(name="b2", bufs=2)),
                "h2": st.enter_context(tc.tile_pool(name="h2", bufs=2)),
                "hc2": st.enter_context(tc.tile_pool(name="hc2", bufs=2)),
                "g4o": st.enter_context(tc.tile_pool(name="g4o", bufs=2)),
                "ps_small": st.enter_context(tc.tile_pool(name="ps_small", bufs=4, space="PSUM")),
                "ps_y": st.enter_context(tc.tile_pool(name="ps_y", bufs=2, space="PSUM")),
            }
            sh = _build_shared(nc, tc, pools)
            f = Dir(nc, pools, sh, params["f"], rev=False, hc_pool=(0, 2, 4, 6))
            b = Dir(nc, pools, sh, params["b"], rev=True, hc_pool=(0, 2, 4, 6))
            f.loads()
            for c in range(NC_D):
                f.uconv_chunk(c)
            f.g2()
            f.scan_prep()
            f.prep(0)
            f.prep(1)
            f.bcast()
            for c in range(NC_D):
                f.z_chunk(c)
            b.loads()
            f.scan_chunk_body(0)
            f.prep(2)
            f.scan_chunk_body(1)
            f.prep(3)
            f.scan_chunk_body(2)
            f.prep(4)
            f.scan_chunk_body(3)
            for c in range(0, 4):
                b.uconv_chunk(c)
            for c in range(0, 4):
                b.z_chunk(c)
            f.prep(5)
            f.scan_chunk_body(4)
            b.uconv_chunk(4)
            b.z_chunk(4)
            f.prep(6)
            f.scan_chunk_body(5)
            b.uconv_chunk(5)
            b.z_chunk(5)
            f.prep(7)
            f.scan_chunk_body(6)
            b.uconv_chunk(6)
            b.z_chunk(6)
            f.scan_chunk_body(7)
            b.uconv_chunk(7)
            b.z_chunk(7)
            b.g2()
            b.scan_prep()
            b.prep(0)
            b.prep(1)
            b.bcast(dve_mod=2)
            f.g4_prep()
            for m in range(NC_T):
                f.g4_block(m)
            b.g4_prep()
            b.scan_chunk_body(0)
            b.prep(2)
            b.scan_chunk_body(1)
            b.prep(3)
            b.scan_chunk_body(2)
            b.prep(4)
            b.scan_chunk_body(3)
            b.prep(5)
            b.scan_chunk_body(4)
            b.prep(6)
            b.scan_chunk_body(5)
            b.prep(7)
            b.scan_chunk_body(6)
            b.scan_chunk_body(7)
            for m in range(NC_T):
                b.g4_block(m)
    _split_excess_waits(nc)
    return nc


def _prep_dir(w):
    bf = ml_dtypes.bfloat16
    in_w, conv_w, conv_b, xp_w, dt_w, dt_b, A_log, Dp, out_w = w
    in_wT = np.asarray(in_w, np.float32).T            # [512, 2048]
    out_wT = np.asarray(out_w, np.float32).T          # [1024, 512]
    xp_wT = np.asarray(xp_w, np.float32).T            # [1024, 64]
    conv_w = np.asarray(conv_w, np.float32)
    conv_b = np.asarray(conv_b, np.float32)
    dt_b = np.asarray(dt_b, np.float32)
    A = -np.exp(np.asarray(A_log, np.float64)).astype(np.float32)
    Dp = np.asarray(Dp, np.float32)

    # inwT packed [P, 4*2048]: k-block k holds in_wT rows k*128..k*128+127
    inwT = in_wT.reshape(4, P, 2 * D_INNER).transpose(1, 0, 2).reshape(P, 4 * 2 * D_INNER)
    # outwT packed [P, 8*512]: c-block holds out_wT rows c*128..
    outwT = out_wT.reshape(NC_D, P, D_MODEL).transpose(1, 0, 2).reshape(P, NC_D * D_MODEL)
    # smallbf [P, 8*64]: xp_wT rows per chunk
    smallbf = xp_wT.reshape(NC_D, P, 64).transpose(1, 0, 2).reshape(P, NC_D * 64)
    # smallf32 [P, 8*18]: per chunk [A(16) | conv_b | dt_b]
    sf = np.concatenate(
        [A.reshape(NC_D, P, D_STATE),
         conv_b.reshape(NC_D, P, 1),
         dt_b.reshape(NC_D, P, 1)], axis=2)
    smallf32 = sf.transpose(1, 0, 2).reshape(P, NC_D * (D_STATE + 2))
    # convdiag [P, 8*4*128]: block (c,k) = diag(conv_w[c*128: , k])
    convdiag = np.zeros((P, NC_D * D_CONV * P), np.float32)
    for c in range(NC_D):
        for k in range(D_CONV):
            blk = (c * D_CONV + k) * P
            convdiag[:, blk:blk + P] = np.diag(conv_w[c * P:(c + 1) * P, k])
    # ddiag [P, 8*128]
    ddiag = np.zeros((P, NC_D * P), np.float32)
    for c in range(NC_D):
        ddiag[:, c * P:(c + 1) * P] = np.diag(Dp[c * P:(c + 1) * P])
    return {
        "inwT": np.ascontiguousarray(inwT).astype(bf),
        "dtwT": np.ascontiguousarray(np.asarray(dt_w).T).astype(bf),
        "outwT": np.ascontiguousarray(outwT).astype(bf),
        "smallbf": np.ascontiguousarray(smallbf).astype(bf),
        "smallf32": np.ascontiguousarray(smallf32),
        "convdiag": np.ascontiguousarray(convdiag).astype(bf),
        "ddiag": np.ascontiguousarray(ddiag).astype(bf),
    }


_CACHED = {}


def kernel(
    x,
    in_w_f, conv_w_f, conv_b_f, xp_w_f, dt_w_f, dt_b_f, A_log_f, D_f, out_w_f,
    in_w_b, conv_w_b, conv_b_b, xp_w_b, dt_w_b, dt_b_b, A_log_b, D_b, out_w_b,
):
    bf = ml_dtypes.bfloat16
    x = np.asarray(x, dtype=np.float32)

    if "nc" not in _CACHED:
        _CACHED["nc"] = build_bass()
    nc = _CACHED["nc"]

    wf = _prep_dir((in_w_f, conv_w_f, conv_b_f, xp_w_f, dt_w_f, dt_b_f,
                    A_log_f, D_f, out_w_f))
    wb = _prep_dir((in_w_b, conv_w_b, conv_b_b, xp_w_b, dt_w_b, dt_b_b,
                    A_log_b, D_b, out_w_b))
    oht = np.kron(np.eye(2 * D_STATE, dtype=np.float32),
                  np.ones((1, P), np.float32)).astype(bf)
    ident = np.eye(P, dtype=np.float32).astype(bf)

    in_maps = []
    for b in range(BATCH):
        m = {"oht": oht, "ident": ident}
        for d, wd in (("f", wf), ("b", wb)):
            for k, v in wd.items():
                m[f"{k}_{d}"] = v
        m["xT"] = np.ascontiguousarray(x[b].T).astype(bf)
        in_maps.append(m)

    res = run_bass_kernel_spmd(nc, in_maps, core_ids=list(range(BATCH)))
    out = np.empty((BATCH, SEQ, D_MODEL), np.float32)
    for b in range(BATCH):
        rb = res.results[b]
        out[b] = rb["out_f"] + rb["out_b"][::-1]
    return out
